# revision 17
# baseline (speedup 1.0000x reference)
"""Trainium2 Bass kernel for nn_AgentPolicy (single-query attention policy net).

Reference computation (B=4096, N=64, FIN=256, D1=512, D2=128):
    x = obs_x @ W1 + b1                        [B, D1]
    y = others @ W1 + b1                       [B, N, D1]
    alpha = (x . y_n) / sqrt(D1)               [B, N]
    beta = softmax(alpha)                      [B, N]
    c = sum_n beta_n y_n                       [B, D1]
    out = concat([x, c])                       [B, 2*D1]
    out1 = softmax(out @ W2 + b2)              [B, D2]
    logits = out1 + NEG * (1 - mask)           [B, D2]
    (value head is dead code)

Algebraic reformulation (avoids materializing y: ~15x fewer flops):
    q = (x @ W1^T) / sqrt(D1)            [B, FIN]
    alpha_n = others_n . q  (+ const/b1 shift, cancelled by softmax)
    c = (beta^T others) @ W1 + b1  (sum beta = 1)
    out @ W2 = x @ W2a + s @ (W1 @ W2b) + b1 @ W2b,  s = beta^T others

The attention core runs in bf16 (validated: unmasked max abs err ~8e-4,
well inside the 2e-2 gate):
  - `others`/`obs_x`/weights are cast fp32->bf16 *during* the DMA (SWDGE
    cast load, measured at full line rate) -- halves SBUF traffic.
  - All hot matmuls are bf16 (1 cyc/row + FWL weight loads).
  - The alpha dot products (DVE scalar_tensor_tensor w/ accum) have no
    DVE fast mode (measured: STT supports none; tensor_tensor only 2x),
    so DVE is budgeted at 1 elem/cyc for them: ~21.6us/tile -- which
    sets the pace together with the ~24us/tile HBM stream.
  - diag(beta) builds are spread across three engines so no single one
    exceeds the DMA pace: gpsimd local_scatter (zeros + writes just the
    diagonal), ACT per-n scalar muls, and DVE broadcast multiplies.
  - PSUM accumulation stays fp32; softmax tail stays fp32.

Sharding: pure data-parallel over B across 8 cores (512 rows/core).
"""

import math

import numpy as np

import concourse.bass as bass
import concourse.mybir as mybir
import concourse.tile as tile
from concourse import bacc
from concourse.bass_utils import run_bass_kernel_spmd
from concourse.masks import make_identity

B, N, FIN, D1, D2 = 4096, 64, 256, 512, 128
NEG = -10000000.0
NCORES = 8
P = 128
KF = FIN // P          # 2 f-chunks of W1 contraction
KD = D1 // P           # 4 d-chunks
NCH = 8                # "others" n's per compute chunk
NCHUNKS = N // NCH     # 8 chunks per row-tile
GRP = 2                # compute chunks per DMA group (tiles >= 1)
NGRP = NCHUNKS // GRP
# diag(beta) build engine per chunk: gpsimd scatter / ACT muls / DVE bcast
DIAG_ENGINE = {0: "act", 1: "gp", 2: "act", 3: "gp",
               4: "act", 5: "gp", 6: "act", 7: "gp"}
F32 = mybir.dt.float32
BF16 = mybir.dt.bfloat16
I16 = mybir.dt.int16
I32 = mybir.dt.int32
AX = mybir.AxisListType
OP = mybir.AluOpType
AF = mybir.ActivationFunctionType


def build_nc(bc):
    """Build the per-core program. bc = batch rows handled by this core."""
    assert bc % P == 0
    rt = bc // P  # number of 128-row tiles
    nc = bacc.Bacc("TRN2")

    obs_d = nc.dram_tensor("obs_x", [bc, FIN], F32, kind="ExternalInput")
    oth_d = nc.dram_tensor("others", [bc, N, FIN], F32, kind="ExternalInput")
    am_d = nc.dram_tensor("action_mask", [bc, D2], I32, kind="ExternalInput")
    w1_d = nc.dram_tensor("W1", [FIN, D1], F32, kind="ExternalInput")
    b1_d = nc.dram_tensor("b1", [D1], F32, kind="ExternalInput")
    w2_d = nc.dram_tensor("W2", [2 * D1, D2], F32, kind="ExternalInput")
    b2_d = nc.dram_tensor("b2", [D2], F32, kind="ExternalInput")
    out_d = nc.dram_tensor("out", [bc, D2], F32, kind="ExternalOutput")

    with tile.TileContext(nc) as tc:
        with (
            tc.tile_pool(name="wpool", bufs=1) as wp,
            tc.tile_pool(name="sb", bufs=3) as sbp,
            tc.tile_pool(name="scr", bufs=3) as scrp,
            tc.tile_pool(name="oth", bufs=3 * NGRP) as othp,
            tc.tile_pool(name="dg", bufs=6) as dgp,
            tc.tile_pool(name="small", bufs=4) as smp,
            tc.tile_pool(name="psx", bufs=1, space="PSUM") as psx,
            tc.tile_pool(name="psq", bufs=1, space="PSUM") as psq,
            tc.tile_pool(name="pst_o", bufs=2, space="PSUM") as pst_o,
            tc.tile_pool(name="pst_s", bufs=1, space="PSUM") as pst_s,
            tc.tile_pool(name="pss", bufs=2, space="PSUM") as pss,
            tc.tile_pool(name="pso", bufs=1, space="PSUM") as pso,
        ):
            # ---------------- one-time setup ----------------
            # SWDGE cast-load order matters (single FIFO): W1 first (the
            # q-chain needs it in ~2us), then tile 0's obs/others chunks,
            # and only then W2/b1 (first needed by tail(0), ~2 tiles in).
            # The rearranged APs must NOT go on HWDGE -- hardware descriptor
            # generation sprays them into 512B segments (measured 16-19us
            # per weight load); SWDGE generates sane descriptors.
            b1_sb = wp.tile([P, KD], F32)           # b1[d] as [128, KD] (ACT bias)
            nc.sync.dma_start(
                b1_sb[:], b1_d.ap().rearrange("(k p) -> p k", p=P))
            b2_sb = wp.tile([1, D2], F32)
            nc.sync.dma_start(b2_sb[:], b2_d.ap().rearrange("(a d) -> a d", a=1))
            w1_sb = wp.tile([P, KF, D1], BF16)      # W1[f, d] bf16, f-chunked
            nc.gpsimd.dma_start(
                w1_sb[:], w1_d.ap().rearrange("(k p) d -> p k d", p=P))
            w2_sb = wp.tile([P, 2 * KD, D2], BF16)  # W2[d, d2] bf16, d-chunked
            b1_bf = wp.tile([P, KD], BF16)          # bf16 b1 for cvec matmul

            def load_w2_b1():
                nc.gpsimd.dma_start(
                    w2_sb[:], w2_d.ap().rearrange("(j p) d -> p j d", p=P))
                nc.gpsimd.dma_start(
                    b1_bf[:], b1_d.ap().rearrange("(k p) -> p k", p=P))

            ident = wp.tile([P, P], F32)
            make_identity(nc, ident[:])
            identb = wp.tile([P, P], BF16)
            nc.scalar.copy(identb[:], ident[:])
            # 0 on the diagonal, -30000 off it: Exp(identNEG + alpha_n)
            # yields diag(exp(alpha_n)) directly (off-diag underflows to 0)
            neg30k = wp.tile([P, 1], F32)
            nc.vector.memset(neg30k[:], -30000.0)
            identneg = wp.tile([P, P], BF16)
            nc.scalar.activation(identneg[:], ident[:], AF.Identity,
                                 bias=neg30k[:], scale=30000.0)

            # idx[p, j] = j*128 + p for the diag local_scatter
            dgidx = wp.tile([P, NCH], I16)
            nc.gpsimd.iota(dgidx[:], pattern=[[P, NCH]], base=0,
                           channel_multiplier=1)

            ones_sb = wp.tile([1, P], F32)
            nc.vector.memset(ones_sb[:], 1.0)

            # W1T[d, f] bf16 (d-chunked) via PE transposes.  Emitted
            # AFTER tile 0's obsT/xt so the PE FIFO runs tile 0's q-chain
            # interleaved with the transposes (shortens the pipeline fill);
            # tile 0's q matmul for chunk kd follows w1t[kd]'s copy.
            w1t_sb = wp.tile([P, KD, FIN], BF16)

            def build_w1t_chunk(kd):
                for kf in range(KF):
                    tp = pst_o.tile([P, P], BF16, tag="pst_o")
                    nc.tensor.transpose(
                        tp[:], w1_sb[:, kf, kd * P:(kd + 1) * P], identb[:]
                    )
                    nc.vector.tensor_copy(
                        w1t_sb[:, kd, kf * P:(kf + 1) * P], tp[:])

            # W12[f, d2] = W1 @ W2b and cvec = b1 @ W2b + b2 -- emitted
            # after alpha(0) so neither its PE chain nor its DVE add can
            # head-of-line-block the first tile's dot products (only
            # tail(0) needs the results).
            w12_sb = wp.tile([P, KF, D2], BF16)
            cvec_sb = wp.tile([1, D2], F32)

            def build_w12_cvec():
                for kf in range(KF):
                    ps = pst_o.tile([P, P], F32, tag="pst_o")
                    for kd in range(KD):
                        nc.tensor.matmul(
                            ps[:, :D2],
                            w1t_sb[:, kd, kf * P:(kf + 1) * P],
                            w2_sb[:, KD + kd, :],
                            start=(kd == 0),
                            stop=(kd == KD - 1),
                        )
                    nc.scalar.copy(w12_sb[:, kf, :], ps[:, :D2])

                cps = pst_o.tile([P, P], F32, tag="pst_o")
                for kd in range(KD):
                    nc.tensor.matmul(
                        cps[:1, :D2],
                        b1_bf[:, kd:kd + 1],
                        w2_sb[:, KD + kd, :],
                        start=(kd == 0),
                        stop=(kd == KD - 1),
                    )
                nc.scalar.copy(cvec_sb[:], cps[:1, :D2])

            # ---------------- pipelined row tiles ----------------
            def prologue(t, q_now=True):
                """Loads + obs^T + xT + q for row-tile t (PE/ACT/DMA).
                others are cast-loaded in groups; tile 0 uses single-chunk
                DMAs so the first dots start ~3us after the weight prefix."""
                r0 = t * P
                st = {}
                obs_t = sbp.tile([P, FIN], BF16, tag="obs", name=f"obs{t}")
                if t == 0:
                    # SWDGE cast load: skips the ACT hop in the fill chain
                    nc.gpsimd.dma_start(obs_t[:], obs_d[r0:r0 + P, :])
                else:
                    # HWDGE + ACT cast: keeps the SWDGE FIFO clear for the
                    # others stream (its completion time gates late tiles)
                    obs_f = sbp.tile([P, FIN], F32, tag="obsf",
                                     name=f"obsf{t}")
                    nc.sync.dma_start(obs_f[:], obs_d[r0:r0 + P, :])
                    nc.gpsimd.tensor_copy(obs_t[:], obs_f[:])
                mask_t = sbp.tile([P, D2], I32, tag="mask", name=f"mask{t}")
                nc.sync.dma_start(mask_t[:], am_d[r0:r0 + P, :])
                st["mask"] = mask_t

                chunks = []
                if t == 0:
                    for c in range(NCHUNKS):
                        oc = othp.tile([P, NCH, FIN], BF16, tag="oth0",
                                       name=f"oc{t}_{c}")
                        nc.gpsimd.dma_start(
                            oc[:], oth_d[r0:r0 + P, c * NCH:(c + 1) * NCH, :])
                        chunks.append(oc[:])
                else:
                    for g in range(NGRP):
                        og = othp.tile([P, GRP * NCH, FIN], BF16, tag="oth",
                                       name=f"og{t}_{g}")
                        nc.gpsimd.dma_start(
                            og[:],
                            oth_d[r0:r0 + P,
                                  g * GRP * NCH:(g + 1) * GRP * NCH, :])
                        for u in range(GRP):
                            chunks.append(og[:, u * NCH:(u + 1) * NCH, :])
                st["oth"] = chunks

                obsT = sbp.tile([P, KF, P], BF16, tag="obsT", name=f"obsT{t}")
                for kf in range(KF):
                    tp = pst_o.tile([P, P], BF16, tag="pst_o")
                    nc.tensor.transpose(
                        tp[:], obs_t[:, kf * P:(kf + 1) * P], identb[:]
                    )
                    nc.scalar.copy(obsT[:, kf, :], tp[:])

                xt_ps = psx.tile([P, KD, P], F32, tag="psx")
                for kd in range(KD):
                    for kf in range(KF):
                        nc.tensor.matmul(
                            xt_ps[:, kd, :],
                            w1_sb[:, kf, kd * P:(kd + 1) * P],
                            obsT[:, kf, :],
                            start=(kf == 0),
                            stop=(kf == KF - 1),
                        )
                xt_sb = sbp.tile([P, KD, P], BF16, tag="xt", name=f"xt{t}")
                for kd in range(KD):
                    nc.scalar.activation(
                        xt_sb[:, kd, :], xt_ps[:, kd, :], AF.Identity,
                        bias=b1_sb[:, kd:kd + 1], scale=1.0,
                    )
                st["xt"] = xt_sb
                if q_now:
                    emit_q(t, st)
                return st

            def emit_q(t, st, kd_mm=None):
                if kd_mm is None:
                    st["q_ps"] = psq.tile([P, FIN], F32, tag="psq",
                                          name=f"q_ps{t}")
                    for kd in range(KD):
                        emit_q(t, st, kd)
                    kd_mm = "fin"
                if kd_mm == "fin":
                    q_sb = sbp.tile([P, FIN], BF16, tag="q", name=f"q{t}")
                    nc.scalar.mul(q_sb[:], st.pop("q_ps")[:],
                                  1.0 / math.sqrt(float(D1)))
                    st["q"] = q_sb
                    return
                nc.tensor.matmul(
                    st["q_ps"][:],
                    st["xt"][:, kd_mm, :],
                    w1t_sb[:, kd_mm, :],
                    start=(kd_mm == 0),
                    stop=(kd_mm == KD - 1),
                )

            def diag_build(t, c, alpha, betau):
                """dgc[b, j, b'] = ident[b, b'] * exp(alpha[b, c*8+j]), bf16."""
                csl = slice(c * NCH, (c + 1) * NCH)
                dgc = dgp.tile([P, NCH, P], BF16, tag="dg", name=f"dg{t}_{c}")
                eng = DIAG_ENGINE[c]
                if eng == "gp":
                    # zeros the tile and writes just the 128 diagonal values
                    nc.gpsimd.local_scatter(
                        dgc[:], betau[:, csl], dgidx[:],
                        channels=P, num_elems=NCH * P, num_idxs=NCH,
                    )
                elif eng == "act":
                    # diag(exp(alpha_n)) straight from fp32 alpha (ACT scale
                    # APs must be fp32, so no betau multiply here).
                    # reversed j: the chunk's 8 matmuls each wait on their
                    # own diag; writing diag j=0 LAST makes MM 0 the gate,
                    # after which MMs 0..7 run back-to-back.
                    for j in reversed(range(NCH)):
                        n = c * NCH + j
                        nc.scalar.activation(
                            dgc[:, j, :], identneg[:], AF.Exp,
                            bias=alpha[:, n:n + 1], scale=1.0,
                        )
                else:
                    # one DVE op for the whole chunk (broadcast trick)
                    nc.vector.tensor_tensor(
                        dgc[:],
                        identb[:].rearrange("p (o b) -> p o b", o=1)
                                 .broadcast_to([P, NCH, P]),
                        betau[:, csl].rearrange("p (n o) -> p n o", o=1)
                                     .broadcast_to([P, NCH, P]),
                        op=OP.mult,
                    )
                return dgc

            def alpha_softmax(t, st):
                """Chunk-pipelined attention core: per 8-n chunk, alpha dot
                products (DVE), exp (ACT, no max subtraction -- alpha is in
                [-11, 11] so fp32-safe; softmax is shift invariant), diag
                builds (gp/act/dve per DIAG_ENGINE) and the weighted-sum
                matmuls (PE, bf16).  DVE-built diags are emitted one chunk
                late so they never head-of-line-block the next chunk's dots
                in the strict DVE FIFO while waiting on exp.  The s
                normalization by 1/sum(exp) happens later on the PSUM
                read-out, so nothing here waits for the full softmax."""
                oth_c, q_sb = st["oth"], st["q"]
                alpha = sbp.tile([P, N], F32, tag="alpha", name=f"al{t}")
                betau = sbp.tile([P, N], BF16, tag="betau", name=f"bu{t}")
                s_ps = pss.tile([P, FIN], F32, tag="pss")
                nmm = [0]
                pending = []

                def flush_chunk(c):
                    dgc = diag_build(t, c, alpha, betau)
                    oc = oth_c[c]
                    for j in range(NCH):
                        nc.tensor.matmul(
                            s_ps[:], dgc[:, j, :], oc[:, j, :],
                            start=(nmm[0] == 0), stop=(nmm[0] == N - 1),
                        )
                        nmm[0] += 1

                for c in range(NCHUNKS):
                    csl = slice(c * NCH, (c + 1) * NCH)
                    oc = oth_c[c]
                    for j in range(NCH):
                        n = c * NCH + j
                        scr = scrp.tile([P, FIN], BF16, tag="scr")
                        nc.vector.scalar_tensor_tensor(
                            out=scr[:],
                            in0=oc[:, j, :],
                            scalar=1.0,
                            in1=q_sb[:],
                            op0=OP.mult,
                            op1=OP.mult,
                            accum_out=alpha[:, n:n + 1],
                        )
                    nc.scalar.activation(
                        betau[:, csl], alpha[:, csl], AF.Exp,
                        bias=0.0, scale=1.0,
                    )
                    if DIAG_ENGINE[c] == "dve":
                        pending.append(c)
                        if len(pending) > 1:
                            flush_chunk(pending.pop(0))
                    else:
                        flush_chunk(c)
                while pending:
                    flush_chunk(pending.pop(0))

                sumexp = smp.tile([P, 1], F32, tag="sumexp")
                nc.vector.reduce_sum(sumexp[:], betau[:], axis=AX.X)
                rbeta = smp.tile([P, 1], F32, tag="rbeta")
                nc.vector.reciprocal(rbeta[:], sumexp[:])
                st["s_ps"], st["rbeta"] = s_ps, rbeta

            def tail_a(t, st):
                """ACT/PE half of the tile tail: s normalization, sT,
                out_pre matmuls, exp+accum.  No DVE ops, so it is emitted
                right after alpha(t) and overlaps the next tile's dots."""
                s_ps, rbeta = st["s_ps"], st["rbeta"]
                xt_sb = st["xt"]

                s_sb = sbp.tile([P, FIN], BF16, tag="s", name=f"s{t}")
                nc.scalar.mul(s_sb[:], s_ps[:], rbeta[:])

                sT = sbp.tile([P, KF, P], BF16, tag="sT", name=f"sT{t}")
                for kf in range(KF):
                    tp = pst_s.tile([P, P], BF16, tag="pst_s")
                    nc.tensor.transpose(
                        tp[:], s_sb[:, kf * P:(kf + 1) * P], identb[:]
                    )
                    nc.scalar.copy(sT[:, kf, :], tp[:])

                o_ps = pso.tile([P, D2], F32, tag="pso")
                nc.tensor.matmul(
                    o_ps[:], ones_sb[:], cvec_sb[:], start=True, stop=False,
                )
                nc.tensor.matmul(
                    o_ps[:], ones_sb[:], b2_sb[:], start=False, stop=False,
                )
                for kd in range(KD):
                    nc.tensor.matmul(
                        o_ps[:], xt_sb[:, kd, :], w2_sb[:, kd, :],
                        start=False, stop=False,
                    )
                for kf in range(KF):
                    nc.tensor.matmul(
                        o_ps[:], sT[:, kf, :], w12_sb[:, kf, :],
                        start=False, stop=(kf == KF - 1),
                    )

                sumexp2 = smp.tile([P, 1], F32, tag="sumexp2")
                exp2 = sbp.tile([P, D2], F32, tag="exp2", name=f"e2{t}")
                nc.scalar.activation(
                    exp2[:], o_ps[:], AF.Exp,
                    bias=0.0, scale=1.0, accum_out=sumexp2[:],
                )
                st["exp2"], st["sumexp2"] = exp2, sumexp2

            def tail_b(t, st):
                """DVE half: softmax2 normalize + mask + store, emitted one
                tile later so the DVE ops' inputs are long ready."""
                r0 = t * P
                rb2 = smp.tile([P, 1], F32, tag="rb2")
                nc.vector.reciprocal(rb2[:], st["sumexp2"][:])
                pre = sbp.tile([P, D2], F32, tag="pre", name=f"pre{t}")
                nc.scalar.mul(pre[:], st["exp2"][:], rb2[:])

                # logits = pre + NEG*(1-mask): add NEG everywhere, then
                # copy back `pre` where mask is nonzero (both on DVE).
                logits = sbp.tile([P, D2], F32, tag="logits", name=f"lg{t}")
                nc.vector.tensor_scalar(
                    out=logits[:], in0=pre[:], scalar1=NEG, scalar2=None,
                    op0=OP.add,
                )
                nc.vector.copy_predicated(logits[:], st["mask"][:], pre[:])
                nc.sync.dma_start(out_d[r0:r0 + P, :], logits[:])

            # pipeline: prologues run TWO tiles ahead so q(t+1) is ready
            # the moment tile t's dots finish, and tail(t) is emitted AFTER
            # alpha(t+1) -- the DVE/ACT queues are strict FIFO, so an
            # eagerly-emitted tail head-of-line-blocks the next tile's dot
            # products while the tail's PE/ACT chain resolves.  Deferring it
            # one tile gives the chain a full tile span to complete.
            states = {0: prologue(0, q_now=False)}
            st0 = states[0]
            st0["q_ps"] = psq.tile([P, FIN], F32, tag="psq", name="q_ps0")
            for kd in range(KD):
                build_w1t_chunk(kd)
                emit_q(0, st0, kd)
            emit_q(0, st0, "fin")
            load_w2_b1()
            states[1] = prologue(1)
            for t in range(rt):
                alpha_softmax(t, states[t])
                if t == 0:
                    build_w12_cvec()
                tail_a(t, states[t])
                if t + 2 < rt:
                    states[t + 2] = prologue(t + 2)
                if t >= 1:
                    tail_b(t - 1, states[t - 1])
                    del states[t - 1]
            tail_b(rt - 1, states[rt - 1])

    nc.finalize()
    return nc


_NC_CACHE = {}


def _get_nc(bc):
    if bc not in _NC_CACHE:
        _NC_CACHE[bc] = build_nc(bc)
    return _NC_CACHE[bc]


def kernel(obs_x, others, action_mask, W1, b1, W2, b2, W3, b3, W4, b4,
           trace=False):
    obs_x = np.ascontiguousarray(np.asarray(obs_x, dtype=np.float32))
    others = np.ascontiguousarray(np.asarray(others, dtype=np.float32))
    action_mask = np.ascontiguousarray(np.asarray(action_mask, dtype=np.int32))
    W1 = np.ascontiguousarray(np.asarray(W1, dtype=np.float32))
    b1 = np.ascontiguousarray(np.asarray(b1, dtype=np.float32))
    W2 = np.ascontiguousarray(np.asarray(W2, dtype=np.float32))
    b2 = np.ascontiguousarray(np.asarray(b2, dtype=np.float32))

    bc = B // NCORES
    nc = _get_nc(bc)
    in_maps = []
    for c in range(NCORES):
        sl = slice(c * bc, (c + 1) * bc)
        in_maps.append({
            "obs_x": obs_x[sl],
            "others": others[sl],
            "action_mask": action_mask[sl],
            "W1": W1, "b1": b1, "W2": W2, "b2": b2,
        })
    res = run_bass_kernel_spmd(nc, in_maps, list(range(NCORES)), trace=trace)
    out = np.concatenate([res.results[c]["out"] for c in range(NCORES)], axis=0)
    if trace:
        return out, res
    return out


# revision 19
# speedup vs baseline: 1.0637x; 1.0637x over previous
"""Trainium2 Bass kernel for nn_AgentPolicy (single-query attention policy net).

Reference computation (B=4096, N=64, FIN=256, D1=512, D2=128):
    x = obs_x @ W1 + b1                        [B, D1]
    y = others @ W1 + b1                       [B, N, D1]
    alpha = (x . y_n) / sqrt(D1)               [B, N]
    beta = softmax(alpha)                      [B, N]
    c = sum_n beta_n y_n                       [B, D1]
    out = concat([x, c])                       [B, 2*D1]
    out1 = softmax(out @ W2 + b2)              [B, D2]
    logits = out1 + NEG * (1 - mask)           [B, D2]
    (value head is dead code)

Algebraic reformulation (avoids materializing y: ~15x fewer flops):
    q = (x @ W1^T) / sqrt(D1)            [B, FIN]
    alpha_n = others_n . q  (+ const/b1 shift, cancelled by softmax)
    c = (beta^T others) @ W1 + b1  (sum beta = 1)
    out @ W2 = x @ W2a + s @ (W1 @ W2b) + b1 @ W2b,  s = beta^T others

The attention core runs in bf16 (validated: unmasked max abs err ~8e-4,
well inside the 2e-2 gate):
  - `others`/`obs_x`/weights are cast fp32->bf16 *during* the DMA (SWDGE
    cast load, measured at full line rate) -- halves SBUF traffic.
  - All hot matmuls are bf16 (1 cyc/row + FWL weight loads).
  - The alpha dot products (DVE scalar_tensor_tensor w/ accum) have no
    DVE fast mode (measured: STT supports none; tensor_tensor only 2x),
    so DVE is budgeted at 1 elem/cyc for them: ~21.6us/tile -- which
    sets the pace together with the ~24us/tile HBM stream.
  - diag(beta) builds stay entirely off the DVE: odd chunks via gpsimd
    local_scatter (zeros the tile, writes just the 128 diagonal values
    from betau), even chunks on ACT via Exp(identNEG + alpha_n) (identNEG
    is 0 on the diagonal, -30000 off it, so off-diagonals underflow to 0
    and the diag IS exp(alpha) -- no separate exp dependency).
  - PSUM accumulation stays fp32; softmax tail stays fp32.
  - Tile tails are split: tail_a (ACT/PE: s-normalize, sT, out matmuls,
    exp+accum) is emitted right after its tile so it overlaps the next
    tile's dots; tail_b (DVE: reciprocal, mask via copy_predicated,
    store) is deferred one tile so it never head-of-line-blocks the DVE
    dot stream.  b2 rides the out-projection as a rank-1 ones x b2
    matmul, keeping setup math off the DVE entirely.

Measured on HW: 174.7us (fp32 baseline) -> 124.0us, rel err 3.1e-12.

Sharding: pure data-parallel over B across 8 cores (512 rows/core).
"""

import math

import numpy as np

import concourse.bass as bass
import concourse.mybir as mybir
import concourse.tile as tile
from concourse import bacc
from concourse.bass_utils import run_bass_kernel_spmd
from concourse.masks import make_identity

B, N, FIN, D1, D2 = 4096, 64, 256, 512, 128
NEG = -10000000.0
NCORES = 8
P = 128
KF = FIN // P          # 2 f-chunks of W1 contraction
KD = D1 // P           # 4 d-chunks
NCH = 8                # "others" n's per compute chunk
NCHUNKS = N // NCH     # 8 chunks per row-tile
GRP = 2                # compute chunks per DMA group (tiles >= 1)
NGRP = NCHUNKS // GRP
# diag(beta) build engine per chunk: gpsimd scatter / ACT muls / DVE bcast
DIAG_ENGINE = {0: "act", 1: "gp", 2: "act", 3: "gp",
               4: "act", 5: "gp", 6: "act", 7: "gp"}
F32 = mybir.dt.float32
BF16 = mybir.dt.bfloat16
I16 = mybir.dt.int16
I32 = mybir.dt.int32
AX = mybir.AxisListType
OP = mybir.AluOpType
AF = mybir.ActivationFunctionType


def build_nc(bc):
    """Build the per-core program. bc = batch rows handled by this core."""
    assert bc % P == 0
    rt = bc // P  # number of 128-row tiles
    nc = bacc.Bacc("TRN2")

    obs_d = nc.dram_tensor("obs_x", [bc, FIN], F32, kind="ExternalInput")
    oth_d = nc.dram_tensor("others", [bc, N, FIN], F32, kind="ExternalInput")
    am_d = nc.dram_tensor("action_mask", [bc, D2], I32, kind="ExternalInput")
    w1_d = nc.dram_tensor("W1", [FIN, D1], F32, kind="ExternalInput")
    b1_d = nc.dram_tensor("b1", [D1], F32, kind="ExternalInput")
    w2_d = nc.dram_tensor("W2", [2 * D1, D2], F32, kind="ExternalInput")
    b2_d = nc.dram_tensor("b2", [D2], F32, kind="ExternalInput")
    out_d = nc.dram_tensor("out", [bc, D2], F32, kind="ExternalOutput")

    with tile.TileContext(nc) as tc:
        with (
            tc.tile_pool(name="wpool", bufs=1) as wp,
            tc.tile_pool(name="sb", bufs=3) as sbp,
            tc.tile_pool(name="scr", bufs=3) as scrp,
            tc.tile_pool(name="oth", bufs=3 * NGRP) as othp,
            tc.tile_pool(name="dg", bufs=6) as dgp,
            tc.tile_pool(name="small", bufs=4) as smp,
            tc.tile_pool(name="psx", bufs=1, space="PSUM") as psx,
            tc.tile_pool(name="psq", bufs=1, space="PSUM") as psq,
            tc.tile_pool(name="pst_o", bufs=2, space="PSUM") as pst_o,
            tc.tile_pool(name="pst_s", bufs=1, space="PSUM") as pst_s,
            tc.tile_pool(name="pss", bufs=2, space="PSUM") as pss,
            tc.tile_pool(name="pso", bufs=1, space="PSUM") as pso,
        ):
            # ---------------- one-time setup ----------------
            # SWDGE cast-load order matters (single FIFO): W1 first (the
            # q-chain needs it in ~2us), then tile 0's obs/others chunks,
            # and only then W2/b1 (first needed by tail(0), ~2 tiles in).
            # The rearranged APs must NOT go on HWDGE -- hardware descriptor
            # generation sprays them into 512B segments (measured 16-19us
            # per weight load); SWDGE generates sane descriptors.
            b1_sb = wp.tile([P, KD], F32)           # b1[d] as [128, KD] (ACT bias)
            nc.sync.dma_start(
                b1_sb[:], b1_d.ap().rearrange("(k p) -> p k", p=P))
            b2_sb = wp.tile([1, D2], F32)
            nc.sync.dma_start(b2_sb[:], b2_d.ap().rearrange("(a d) -> a d", a=1))
            w1_sb = wp.tile([P, KF, D1], BF16)      # W1[f, d] bf16, f-chunked
            nc.gpsimd.dma_start(
                w1_sb[:], w1_d.ap().rearrange("(k p) d -> p k d", p=P))
            w2_sb = wp.tile([P, 2 * KD, D2], BF16)  # W2[d, d2] bf16, d-chunked
            b1_bf = wp.tile([P, KD], BF16)          # bf16 b1 for cvec matmul

            def load_w2_b1():
                nc.gpsimd.dma_start(
                    w2_sb[:], w2_d.ap().rearrange("(j p) d -> p j d", p=P))
                nc.gpsimd.dma_start(
                    b1_bf[:], b1_d.ap().rearrange("(k p) -> p k", p=P))

            ident = wp.tile([P, P], F32)
            make_identity(nc, ident[:])
            identb = wp.tile([P, P], BF16)
            nc.scalar.copy(identb[:], ident[:])
            # 0 on the diagonal, -30000 off it: Exp(identNEG + alpha_n)
            # yields diag(exp(alpha_n)) directly (off-diag underflows to 0)
            neg30k = wp.tile([P, 1], F32)
            nc.vector.memset(neg30k[:], -30000.0)
            identneg = wp.tile([P, P], BF16)
            nc.scalar.activation(identneg[:], ident[:], AF.Identity,
                                 bias=neg30k[:], scale=30000.0)

            # idx[p, j] = j*128 + p for the diag local_scatter
            dgidx = wp.tile([P, NCH], I16)
            nc.gpsimd.iota(dgidx[:], pattern=[[P, NCH]], base=0,
                           channel_multiplier=1)

            ones_sb = wp.tile([1, P], F32)
            nc.vector.memset(ones_sb[:], 1.0)

            # W1T[d, f] bf16 (d-chunked) via PE transposes (PSUM->SBUF
            # copies on DVE: it is idle during the fill, and this keeps the
            # fill-critical ACT chain short)
            w1t_sb = wp.tile([P, KD, FIN], BF16)
            for kd in range(KD):
                for kf in range(KF):
                    tp = pst_o.tile([P, P], BF16, tag="pst_o")
                    nc.tensor.transpose(
                        tp[:], w1_sb[:, kf, kd * P:(kd + 1) * P], identb[:]
                    )
                    nc.vector.tensor_copy(
                        w1t_sb[:, kd, kf * P:(kf + 1) * P], tp[:])

            # W12[f, d2] = W1 @ W2b and cvec = b1 @ W2b + b2 -- emitted
            # after alpha(0) so neither its PE chain nor its DVE add can
            # head-of-line-block the first tile's dot products (only
            # tail(0) needs the results).
            w12_sb = wp.tile([P, KF, D2], BF16)
            cvec_sb = wp.tile([1, D2], F32)

            def build_w12_cvec():
                for kf in range(KF):
                    ps = pst_o.tile([P, P], F32, tag="pst_o")
                    for kd in range(KD):
                        nc.tensor.matmul(
                            ps[:, :D2],
                            w1t_sb[:, kd, kf * P:(kf + 1) * P],
                            w2_sb[:, KD + kd, :],
                            start=(kd == 0),
                            stop=(kd == KD - 1),
                        )
                    nc.scalar.copy(w12_sb[:, kf, :], ps[:, :D2])

                cps = pst_o.tile([P, P], F32, tag="pst_o")
                for kd in range(KD):
                    nc.tensor.matmul(
                        cps[:1, :D2],
                        b1_bf[:, kd:kd + 1],
                        w2_sb[:, KD + kd, :],
                        start=(kd == 0),
                        stop=(kd == KD - 1),
                    )
                nc.scalar.copy(cvec_sb[:], cps[:1, :D2])

            # ---------------- pipelined row tiles ----------------
            def prologue(t):
                """Loads + obs^T + xT + q for row-tile t (PE/ACT/DMA).
                others are cast-loaded in groups; tile 0 uses single-chunk
                DMAs so the first dots start ~3us after the weight prefix."""
                r0 = t * P
                st = {}
                obs_t = sbp.tile([P, FIN], BF16, tag="obs", name=f"obs{t}")
                if t == 0:
                    # SWDGE cast load: skips the ACT hop in the fill chain
                    nc.gpsimd.dma_start(obs_t[:], obs_d[r0:r0 + P, :])
                else:
                    # HWDGE + ACT cast: keeps the SWDGE FIFO clear for the
                    # others stream (its completion time gates late tiles)
                    obs_f = sbp.tile([P, FIN], F32, tag="obsf",
                                     name=f"obsf{t}")
                    nc.sync.dma_start(obs_f[:], obs_d[r0:r0 + P, :])
                    nc.gpsimd.tensor_copy(obs_t[:], obs_f[:])
                mask_t = sbp.tile([P, D2], I32, tag="mask", name=f"mask{t}")
                nc.sync.dma_start(mask_t[:], am_d[r0:r0 + P, :])
                st["mask"] = mask_t

                chunks = []
                if t == 0:
                    for c in range(NCHUNKS):
                        oc = othp.tile([P, NCH, FIN], BF16, tag="oth0",
                                       name=f"oc{t}_{c}")
                        nc.gpsimd.dma_start(
                            oc[:], oth_d[r0:r0 + P, c * NCH:(c + 1) * NCH, :])
                        chunks.append(oc[:])
                else:
                    for g in range(NGRP):
                        og = othp.tile([P, GRP * NCH, FIN], BF16, tag="oth",
                                       name=f"og{t}_{g}")
                        nc.gpsimd.dma_start(
                            og[:],
                            oth_d[r0:r0 + P,
                                  g * GRP * NCH:(g + 1) * GRP * NCH, :])
                        for u in range(GRP):
                            chunks.append(og[:, u * NCH:(u + 1) * NCH, :])
                st["oth"] = chunks

                obsT = sbp.tile([P, KF, P], BF16, tag="obsT", name=f"obsT{t}")
                for kf in range(KF):
                    tp = pst_o.tile([P, P], BF16, tag="pst_o")
                    nc.tensor.transpose(
                        tp[:], obs_t[:, kf * P:(kf + 1) * P], identb[:]
                    )
                    nc.scalar.copy(obsT[:, kf, :], tp[:])

                xt_ps = psx.tile([P, KD, P], F32, tag="psx")
                for kd in range(KD):
                    for kf in range(KF):
                        nc.tensor.matmul(
                            xt_ps[:, kd, :],
                            w1_sb[:, kf, kd * P:(kd + 1) * P],
                            obsT[:, kf, :],
                            start=(kf == 0),
                            stop=(kf == KF - 1),
                        )
                xt_sb = sbp.tile([P, KD, P], BF16, tag="xt", name=f"xt{t}")
                for kd in range(KD):
                    nc.scalar.activation(
                        xt_sb[:, kd, :], xt_ps[:, kd, :], AF.Identity,
                        bias=b1_sb[:, kd:kd + 1], scale=1.0,
                    )
                st["xt"] = xt_sb

                q_ps = psq.tile([P, FIN], F32, tag="psq")
                for kd in range(KD):
                    nc.tensor.matmul(
                        q_ps[:],
                        xt_sb[:, kd, :],
                        w1t_sb[:, kd, :],
                        start=(kd == 0),
                        stop=(kd == KD - 1),
                    )
                q_sb = sbp.tile([P, FIN], BF16, tag="q", name=f"q{t}")
                nc.scalar.mul(q_sb[:], q_ps[:], 1.0 / math.sqrt(float(D1)))
                st["q"] = q_sb
                return st

            def diag_build(t, c, alpha, betau):
                """dgc[b, j, b'] = ident[b, b'] * exp(alpha[b, c*8+j]), bf16."""
                csl = slice(c * NCH, (c + 1) * NCH)
                dgc = dgp.tile([P, NCH, P], BF16, tag="dg", name=f"dg{t}_{c}")
                eng = DIAG_ENGINE[c]
                if eng == "gp":
                    # zeros the tile and writes just the 128 diagonal values
                    nc.gpsimd.local_scatter(
                        dgc[:], betau[:, csl], dgidx[:],
                        channels=P, num_elems=NCH * P, num_idxs=NCH,
                    )
                elif eng == "act":
                    # diag(exp(alpha_n)) straight from fp32 alpha (ACT scale
                    # APs must be fp32, so no betau multiply here).
                    # reversed j: the chunk's 8 matmuls each wait on their
                    # own diag; writing diag j=0 LAST makes MM 0 the gate,
                    # after which MMs 0..7 run back-to-back.
                    for j in reversed(range(NCH)):
                        n = c * NCH + j
                        nc.scalar.activation(
                            dgc[:, j, :], identneg[:], AF.Exp,
                            bias=alpha[:, n:n + 1], scale=1.0,
                        )
                else:
                    # one DVE op for the whole chunk (broadcast trick)
                    nc.vector.tensor_tensor(
                        dgc[:],
                        identb[:].rearrange("p (o b) -> p o b", o=1)
                                 .broadcast_to([P, NCH, P]),
                        betau[:, csl].rearrange("p (n o) -> p n o", o=1)
                                     .broadcast_to([P, NCH, P]),
                        op=OP.mult,
                    )
                return dgc

            def alpha_softmax(t, st):
                """Chunk-pipelined attention core: per 8-n chunk, alpha dot
                products (DVE), exp (ACT, no max subtraction -- alpha is in
                [-11, 11] so fp32-safe; softmax is shift invariant), diag
                builds (gp/act/dve per DIAG_ENGINE) and the weighted-sum
                matmuls (PE, bf16).  DVE-built diags are emitted one chunk
                late so they never head-of-line-block the next chunk's dots
                in the strict DVE FIFO while waiting on exp.  The s
                normalization by 1/sum(exp) happens later on the PSUM
                read-out, so nothing here waits for the full softmax."""
                oth_c, q_sb = st["oth"], st["q"]
                alpha = sbp.tile([P, N], F32, tag="alpha", name=f"al{t}")
                betau = sbp.tile([P, N], BF16, tag="betau", name=f"bu{t}")
                s_ps = pss.tile([P, FIN], F32, tag="pss")
                nmm = [0]
                pending = []

                def flush_chunk(c):
                    dgc = diag_build(t, c, alpha, betau)
                    oc = oth_c[c]
                    for j in range(NCH):
                        nc.tensor.matmul(
                            s_ps[:], dgc[:, j, :], oc[:, j, :],
                            start=(nmm[0] == 0), stop=(nmm[0] == N - 1),
                        )
                        nmm[0] += 1

                for c in range(NCHUNKS):
                    csl = slice(c * NCH, (c + 1) * NCH)
                    oc = oth_c[c]
                    for j in range(NCH):
                        n = c * NCH + j
                        scr = scrp.tile([P, FIN], BF16, tag="scr")
                        nc.vector.scalar_tensor_tensor(
                            out=scr[:],
                            in0=oc[:, j, :],
                            scalar=1.0,
                            in1=q_sb[:],
                            op0=OP.mult,
                            op1=OP.mult,
                            accum_out=alpha[:, n:n + 1],
                        )
                    nc.scalar.activation(
                        betau[:, csl], alpha[:, csl], AF.Exp,
                        bias=0.0, scale=1.0,
                    )
                    if DIAG_ENGINE[c] == "dve":
                        pending.append(c)
                        if len(pending) > 1:
                            flush_chunk(pending.pop(0))
                    else:
                        flush_chunk(c)
                while pending:
                    flush_chunk(pending.pop(0))

                sumexp = smp.tile([P, 1], F32, tag="sumexp")
                nc.vector.reduce_sum(sumexp[:], betau[:], axis=AX.X)
                rbeta = smp.tile([P, 1], F32, tag="rbeta")
                nc.vector.reciprocal(rbeta[:], sumexp[:])
                st["s_ps"], st["rbeta"] = s_ps, rbeta

            def tail_a(t, st):
                """ACT/PE half of the tile tail: s normalization, sT,
                out_pre matmuls, exp+accum.  No DVE ops, so it is emitted
                right after alpha(t) and overlaps the next tile's dots."""
                s_ps, rbeta = st["s_ps"], st["rbeta"]
                xt_sb = st["xt"]

                s_sb = sbp.tile([P, FIN], BF16, tag="s", name=f"s{t}")
                nc.scalar.mul(s_sb[:], s_ps[:], rbeta[:])

                sT = sbp.tile([P, KF, P], BF16, tag="sT", name=f"sT{t}")
                for kf in range(KF):
                    tp = pst_s.tile([P, P], BF16, tag="pst_s")
                    nc.tensor.transpose(
                        tp[:], s_sb[:, kf * P:(kf + 1) * P], identb[:]
                    )
                    nc.scalar.copy(sT[:, kf, :], tp[:])

                o_ps = pso.tile([P, D2], F32, tag="pso")
                nc.tensor.matmul(
                    o_ps[:], ones_sb[:], cvec_sb[:], start=True, stop=False,
                )
                nc.tensor.matmul(
                    o_ps[:], ones_sb[:], b2_sb[:], start=False, stop=False,
                )
                for kd in range(KD):
                    nc.tensor.matmul(
                        o_ps[:], xt_sb[:, kd, :], w2_sb[:, kd, :],
                        start=False, stop=False,
                    )
                for kf in range(KF):
                    nc.tensor.matmul(
                        o_ps[:], sT[:, kf, :], w12_sb[:, kf, :],
                        start=False, stop=(kf == KF - 1),
                    )

                sumexp2 = smp.tile([P, 1], F32, tag="sumexp2")
                exp2 = sbp.tile([P, D2], F32, tag="exp2", name=f"e2{t}")
                nc.scalar.activation(
                    exp2[:], o_ps[:], AF.Exp,
                    bias=0.0, scale=1.0, accum_out=sumexp2[:],
                )
                st["exp2"], st["sumexp2"] = exp2, sumexp2

            def tail_b(t, st):
                """DVE half: softmax2 normalize + mask + store, emitted one
                tile later so the DVE ops' inputs are long ready."""
                r0 = t * P
                rb2 = smp.tile([P, 1], F32, tag="rb2")
                nc.vector.reciprocal(rb2[:], st["sumexp2"][:])
                pre = sbp.tile([P, D2], F32, tag="pre", name=f"pre{t}")
                nc.scalar.mul(pre[:], st["exp2"][:], rb2[:])

                # logits = pre + NEG*(1-mask): add NEG everywhere, then
                # copy back `pre` where mask is nonzero (both on DVE).
                logits = sbp.tile([P, D2], F32, tag="logits", name=f"lg{t}")
                nc.vector.tensor_scalar(
                    out=logits[:], in0=pre[:], scalar1=NEG, scalar2=None,
                    op0=OP.add,
                )
                nc.vector.copy_predicated(logits[:], st["mask"][:], pre[:])
                nc.sync.dma_start(out_d[r0:r0 + P, :], logits[:])

            # pipeline: prologues run TWO tiles ahead so q(t+1) is ready
            # the moment tile t's dots finish, and tail(t) is emitted AFTER
            # alpha(t+1) -- the DVE/ACT queues are strict FIFO, so an
            # eagerly-emitted tail head-of-line-blocks the next tile's dot
            # products while the tail's PE/ACT chain resolves.  Deferring it
            # one tile gives the chain a full tile span to complete.
            states = {0: prologue(0)}
            load_w2_b1()
            states[1] = prologue(1)
            for t in range(rt):
                alpha_softmax(t, states[t])
                if t == 0:
                    build_w12_cvec()
                tail_a(t, states[t])
                if t + 2 < rt:
                    states[t + 2] = prologue(t + 2)
                if t >= 1:
                    tail_b(t - 1, states[t - 1])
                    del states[t - 1]
            tail_b(rt - 1, states[rt - 1])

    nc.finalize()
    return nc


_NC_CACHE = {}


def _get_nc(bc):
    if bc not in _NC_CACHE:
        _NC_CACHE[bc] = build_nc(bc)
    return _NC_CACHE[bc]


def kernel(obs_x, others, action_mask, W1, b1, W2, b2, W3, b3, W4, b4,
           trace=False):
    obs_x = np.ascontiguousarray(np.asarray(obs_x, dtype=np.float32))
    others = np.ascontiguousarray(np.asarray(others, dtype=np.float32))
    action_mask = np.ascontiguousarray(np.asarray(action_mask, dtype=np.int32))
    W1 = np.ascontiguousarray(np.asarray(W1, dtype=np.float32))
    b1 = np.ascontiguousarray(np.asarray(b1, dtype=np.float32))
    W2 = np.ascontiguousarray(np.asarray(W2, dtype=np.float32))
    b2 = np.ascontiguousarray(np.asarray(b2, dtype=np.float32))

    bc = B // NCORES
    nc = _get_nc(bc)
    in_maps = []
    for c in range(NCORES):
        sl = slice(c * bc, (c + 1) * bc)
        in_maps.append({
            "obs_x": obs_x[sl],
            "others": others[sl],
            "action_mask": action_mask[sl],
            "W1": W1, "b1": b1, "W2": W2, "b2": b2,
        })
    res = run_bass_kernel_spmd(nc, in_maps, list(range(NCORES)), trace=trace)
    out = np.concatenate([res.results[c]["out"] for c in range(NCORES)], axis=0)
    if trace:
        return out, res
    return out
